# revision 9
# baseline (speedup 1.0000x reference)
"""Multi-Head Latent Attention Trainium2 kernel (8-core data parallel).

Sharding: pure data parallel over (batch=4) x (sequence halves=2) = 8 cores.
Each core computes the full attention output for its 1024 query tokens of one
batch, using all 2048 keys/values of that batch (KV computed redundantly per
batch pair — negligible cost). No collectives.

Per-core kernel (all matmuls in float32r: fp32 memory, 1 cyc/row on PE):
  1. latent.T [128, 2048] = [w_kv_a; w_q_a] @ x.T   (fused both paths)
  2. rmsnorm along partitions: sum-of-squares via ones-matmul,
     inv_rms = exp(-0.5*ln(ms+eps)) on ACT, broadcast via K=2 matmul.
  3. per head-pair (A=2hp rows 0-63, B=2hp+1 rows 64-127):
     K.T_pair/Q.T_pair via row-group-concurrent matmuls, V_pair token-major
     with ones column (softmax denominators for free).
     S^T = K.T_tile.T @ Q.T chunk (A/B row-group concurrent, shared 2-bank
     psum tile), P = exp(S*scale) one ACT op per [128,1024] tile,
     y_aug.T accumulated over k-tiles (M=65 incl. denominator row).
     Normalize: DVE reciprocal + K=1 broadcast matmul, applied at y.T level.
  4. proj: y.T chunks as lhsT against w_proj.T, accumulate over 16 heads.

Token order per core: [own 1024 queries, other half] so the SPMD NEFF always
reads queries at offset 0 (K/V order irrelevant to softmax).
"""
import numpy as np

import concourse.bacc as bacc
import concourse.bass as bass
import concourse.mybir as mybir
import concourse.tile as tile
from concourse.bass_utils import run_bass_kernel_spmd

F32 = mybir.dt.float32
F32R = mybir.dt.float32r
AF = mybir.ActivationFunctionType

B, N, C = 4, 2048, 1024
H, D, R = 16, 64, 64
NT = 2048          # kv tokens per core (full batch sequence)
NQ = 1024          # query tokens per core
EPS = 1e-6
SCALE = D ** -0.5
N_CORES = 8


def build_nc(reps: int = 1):
    nc = bacc.Bacc("TRN2", target_bir_lowering=False)
    x_t = nc.dram_tensor("x_t", [C, NT], F32R, kind="ExternalInput")
    wa_t = nc.dram_tensor("wa_t", [C, 2 * R], F32R, kind="ExternalInput")
    wqk_t = nc.dram_tensor("wqk_t", [2 * R, H * D], F32R, kind="ExternalInput")
    wv_t = nc.dram_tensor("wv_t", [R, H * D], F32R, kind="ExternalInput")
    wp_t = nc.dram_tensor("wp_t", [H * D, C], F32R, kind="ExternalInput")
    ones2_t = nc.dram_tensor("ones2_t", [128, 2], F32R, kind="ExternalInput")
    y_out = nc.dram_tensor("y_out", [NQ, C], F32, kind="ExternalOutput")
    out_r = y_out.rearrange("(qt p) c -> p qt c", p=128)

    with tile.TileContext(nc) as tc:
        with (
            tc.tile_pool(name="wsb", bufs=1) as wsb,
            tc.tile_pool(name="res", bufs=1) as res,
            tc.tile_pool(name="xs", bufs=2) as xs,
            tc.tile_pool(name="work", bufs=2) as work,
            tc.tile_pool(name="pts", bufs=3) as pts,
            tc.tile_pool(name="small", bufs=2) as small,
            tc.tile_pool(name="drp", bufs=2, space="DRAM") as drp,
        ):

            def bcast_ap(dram_row, n_part):
                return bass.AP(tensor=dram_row.tensor, offset=dram_row.offset,
                               ap=[[0, n_part]] + list(dram_row.ap[1:]))

            import contextlib

            def loop_ctx():
                if reps > 1:
                    return tc.For_i(0, reps, 1)
                return contextlib.nullcontext()

            with loop_ctx():
                # ---- weights ----
                wa_sb = wsb.tile([128, 8, 2 * R], F32R, tag="wa")
                for kc in range(8):
                    nc.sync.dma_start(wa_sb[:, kc, :], wa_t[kc * 128:(kc + 1) * 128, :])
                wqk_sb = wsb.tile([128, H * D], F32R, tag="wqk")
                nc.sync.dma_start(wqk_sb[:], wqk_t[:])
                wv_sb = wsb.tile([128, H * D], F32R, tag="wv")
                nc.sync.dma_start(wv_sb[0:64, :], wv_t[:])
                nc.sync.dma_start(wv_sb[64:128, :], wv_t[:])
                wp_sb = wsb.tile([128, 8, C], F32R, tag="wp")
                for hp in range(8):
                    nc.sync.dma_start(wp_sb[:, hp, :], wp_t[hp * 128:(hp + 1) * 128, :])

                # ---- constants ----
                ones_f32 = wsb.tile([128, 1], F32, tag="ones")
                nc.vector.memset(ones_f32[:], 1.0)
                eps2 = wsb.tile([2, 1], F32, tag="eps")
                nc.vector.memset(eps2[:], EPS)
                ones2 = wsb.tile([128, 2], F32R, tag="ones2")
                nc.sync.dma_start(ones2[:], ones2_t[:])

                # ---- resident tensors ----
                lat_n = res.tile([128, NT], F32R, tag="lat_n")
                ybuf = res.tile([128, 8, NQ], F32R, tag="ybuf")

                # ---- phase 0: fused latents + rmsnorm ----
                with tc.tile_pool(name="ps0", bufs=2, space="PSUM") as ps0:
                    lat_ps = ps0.tile([128, NT], F32, tag="lat", bufs=1)
                    for kc in range(8):
                        xt = xs.tile([128, NT], F32R, tag="x")
                        nc.sync.dma_start(xt[:], x_t[kc * 128:(kc + 1) * 128, :])
                        for t4 in range(4):
                            nc.tensor.matmul(
                                lat_ps[:, t4 * 512:(t4 + 1) * 512],
                                wa_sb[:, kc, :],
                                xt[:, t4 * 512:(t4 + 1) * 512],
                                start=(kc == 0), stop=(kc == 7))
                    for t4 in range(4):
                        sl = bass.ts(t4, 512)
                        sq = small.tile([128, 512], F32R, tag="sq")
                        nc.scalar.activation(sq[:], lat_ps[:, sl], AF.Square)
                        ssq = ps0.tile([2, 512], F32, tag="aux")
                        nc.tensor.matmul(ssq[:], ones2[:], sq[:], start=True, stop=True)
                        lns = small.tile([2, 512], F32, tag="lns")
                        nc.scalar.activation(lns[:], ssq[:], AF.Ln,
                                             bias=eps2[:], scale=1.0 / R)
                        inv = small.tile([2, 512], F32R, tag="lns")
                        nc.scalar.activation(inv[:], lns[:], AF.Exp, scale=-0.5)
                        inv_d = drp.tile([2, 512], F32R, tag="inv_d")
                        nc.sync.dma_start(inv_d[:], inv[:])
                        bc_sb = small.tile([128, 512], F32R, tag="bc_sb")
                        nc.sync.dma_start(bc_sb[0:64, :], bcast_ap(inv_d[0:1, :], 64))
                        nc.sync.dma_start(bc_sb[64:128, :],
                                          bcast_ap(inv_d[1:2, :], 64))
                        nc.vector.tensor_mul(lat_n[:, sl], lat_ps[:, sl], bc_sb[:])

                # ---- phase 1: head pairs ----
                # duplicate kv-latent at partitions 64-127 (row-group pairing
                # for V matmuls)
                lat_kv2 = res.tile([128, NT], F32R, tag="lat_kv2")
                nc.sync.dma_start(lat_kv2[64:128, :], lat_n[0:64, :])

                with (
                    tc.tile_pool(name="pst", bufs=2, space="PSUM") as pst,
                    tc.tile_pool(name="psy", bufs=2, space="PSUM") as psy,
                ):
                    def kqv_tiles_and_thunks(hp):
                        """Allocate K.T/Q.T/V tiles for pair hp and return a list
                        of emission thunks (matmul+evac units) to interleave."""
                        hsl = bass.ts(hp, 128)
                        ktp = work.tile([128, NT], F32R, tag="ktp",
                                        name=f"ktp{hp}")
                        qtp = work.tile([128, NQ], F32R, tag="qtp",
                                        name=f"qtp{hp}")
                        vt = work.tile([128, 16, 130], F32R, tag="vt",
                                       name=f"vt{hp}")
                        vt2 = vt.rearrange("p k (s u) -> p k s u", s=2)
                        thunks = []

                        def k_unit(t4):
                            sl = bass.ts(t4, 512)
                            kps = pst.tile([128, 512], F32, tag="st")
                            nc.tensor.matmul(kps[:], wqk_sb[0:64, hsl],
                                             lat_n[0:64, sl],
                                             start=True, stop=True)
                            if t4 < 2:
                                qps = pst.tile([128, 512], F32, tag="st")
                                nc.tensor.matmul(qps[:], wqk_sb[64:128, hsl],
                                                 lat_n[64:128, sl],
                                                 start=True, stop=True)
                                nc.vector.tensor_copy(qtp[:, sl], qps[:])
                            nc.vector.tensor_copy(ktp[:, sl], kps[:])

                        def v_unit(kt0):
                            # two V k-tiles (even via lat_n rows 0-63, odd via
                            # lat_kv2 rows 64-127 — concurrent row groups)
                            for kt in (kt0, kt0 + 1):
                                vps = pst.tile([128, 128], F32, tag="st")
                                if kt % 2 == 0:
                                    nc.tensor.matmul(
                                        vps[:], lat_n[0:64, bass.ts(kt, 128)],
                                        wv_sb[0:64, hsl], start=True, stop=True)
                                else:
                                    nc.tensor.matmul(
                                        vps[:], lat_kv2[64:128, bass.ts(kt, 128)],
                                        wv_sb[64:128, hsl], start=True, stop=True)
                                nc.vector.tensor_copy(
                                    vt2[:, kt, :, 0:64],
                                    vps[:].rearrange("p (s u) -> p s u", s=2))

                        def ones_unit():
                            nc.vector.tensor_copy(
                                vt2[:, :, :, 64:65],
                                ones_f32[:].broadcast_to([128, 16, 2, 1]))

                        for t4 in range(4):
                            thunks.append(lambda t4=t4: k_unit(t4))
                        thunks.append(ones_unit)
                        for kt0 in range(0, 16, 2):
                            thunks.append(lambda kt0=kt0: v_unit(kt0))
                        return (ktp, qtp, vt), thunks

                    cur_tiles, cur_thunks = kqv_tiles_and_thunks(0)
                    for th in cur_thunks:
                        th()
                    pending = []
                    for hp in range(8):
                        ktp, qtp, vt = cur_tiles
                        if hp < 7:
                            cur_tiles, pending = kqv_tiles_and_thunks(hp + 1)
                        else:
                            pending = []
                        # attention per query chunk: 32 S^T segments (kt x A/B)
                        # streamed through 3-bank psum groups, one exp per group
                        for qc in range(2):
                            qsl = bass.ts(qc, 512)
                            ya = psy.tile([65, 512], F32, tag="y")
                            yb = psy.tile([65, 512], F32, tag="y")
                            n_seg = 32
                            s = 0
                            while s < n_seg:
                                gw = min(3, n_seg - s)
                                st = pst.tile([128, 1536], F32, tag="st")
                                for j in range(gw):
                                    kt, head = (s + j) // 2, (s + j) % 2
                                    ksl = bass.ts(kt, 128)
                                    osl = bass.ts(j, 512)
                                    if head == 0:
                                        nc.tensor.matmul(st[:, osl], ktp[0:64, ksl],
                                                         qtp[0:64, qsl],
                                                         start=True, stop=True)
                                    else:
                                        nc.tensor.matmul(st[:, osl], ktp[64:128, ksl],
                                                         qtp[64:128, qsl],
                                                         start=True, stop=True)
                                pt = pts.tile([128, 1536], F32R, tag="pt")
                                nc.scalar.activation(pt[:, 0:gw * 512],
                                                     st[:, 0:gw * 512],
                                                     AF.Exp, scale=SCALE)
                                for j in range(gw):
                                    kt, head = (s + j) // 2, (s + j) % 2
                                    osl = bass.ts(j, 512)
                                    if head == 0:
                                        nc.tensor.matmul(ya[:], vt[:, kt, 0:65],
                                                         pt[:, osl],
                                                         start=(kt == 0),
                                                         stop=(kt == 15))
                                    else:
                                        nc.tensor.matmul(yb[:], vt[:, kt, 65:130],
                                                         pt[:, osl],
                                                         start=(kt == 0),
                                                         stop=(kt == 15))
                                s += gw
                                # interleave next pair's K/Q/V production
                                if pending:
                                    pending.pop(0)()
                            # normalize + write into ybuf
                            for half, yp in ((0, ya), (1, yb)):
                                ysb = small.tile([65, 512], F32, tag="ysb")
                                nc.vector.tensor_copy(ysb[:], yp[:])
                                rq = small.tile([1, 512], F32R, tag="rq")
                                with nc.allow_low_precision(
                                        reason="f32r softmax denominators"):
                                    nc.vector.reciprocal(rq[:], ysb[64:65, :])
                                rq_d = drp.tile([1, 512], F32R, tag="rq_d")
                                nc.sync.dma_start(rq_d[:], rq[:])
                                bcy = small.tile([64, 512], F32R, tag="bcy")
                                nc.sync.dma_start(bcy[:], bcast_ap(rq_d[0:1, :], 64))
                                if half == 0:
                                    nc.vector.tensor_mul(ybuf[0:64, hp, qsl],
                                                         ysb[0:64, :], bcy[:])
                                else:
                                    y2b = small.tile([64, 512], F32R, tag="y2b")
                                    nc.vector.tensor_mul(y2b[:], ysb[0:64, :],
                                                         bcy[:])
                                    # partition shift 0-63 -> 64-127 via DMA
                                    nc.sync.dma_start(ybuf[64:128, hp, qsl], y2b[:])
                        for th in pending:
                            th()
                    # ---- proj ----
                    for qt in range(8):
                        for cc in range(2):
                            pj = pst.tile([128, 512], F32, tag="st")
                            for hp2 in range(8):
                                nc.tensor.matmul(
                                    pj[:], ybuf[:, hp2, bass.ts(qt, 128)],
                                    wp_sb[:, hp2, bass.ts(cc, 512)],
                                    start=(hp2 == 0), stop=(hp2 == 7))
                            osb = small.tile([128, 512], F32, tag="osb")
                            nc.vector.tensor_copy(osb[:], pj[:])
                            nc.sync.dma_start(out_r[:, qt, bass.ts(cc, 512)], osb[:])
    nc.compile()
    return nc


def prep_inputs(x, w_kv_a, w_kv_b, w_q_a, w_q_b, w_proj, kv_norm_w, q_norm_w):
    """Host-side sharding/layout prep. Returns per-core input maps."""
    x = np.asarray(x, dtype=np.float32)
    w_kv_b = np.asarray(w_kv_b, dtype=np.float32) * np.asarray(kv_norm_w, np.float32)[None, :]
    w_q_b = np.asarray(w_q_b, dtype=np.float32) * np.asarray(q_norm_w, np.float32)[None, :]
    wa_t = np.ascontiguousarray(
        np.concatenate([np.asarray(w_kv_a, np.float32),
                        np.asarray(w_q_a, np.float32)], axis=0).T)   # [C, 128]
    kvb = w_kv_b.reshape(H, 2, D, R)
    wk_t = np.ascontiguousarray(kvb[:, 0].transpose(2, 0, 1).reshape(R, H * D))
    wv_t = np.ascontiguousarray(kvb[:, 1].transpose(2, 0, 1).reshape(R, H * D))
    wq_t = np.ascontiguousarray(w_q_b.T)                              # [R, H*D]
    wqk_t = np.ascontiguousarray(np.concatenate([wk_t, wq_t], axis=0))
    wp_t = np.ascontiguousarray(np.asarray(w_proj, np.float32).T)     # [H*D, C]

    in_maps = []
    for core in range(N_CORES):
        b, half = divmod(core, 2)
        own = x[b, half * NQ:(half + 1) * NQ]
        other = x[b, (1 - half) * NQ:(2 - half) * NQ]
        x_perm_t = np.ascontiguousarray(np.concatenate([own, other], axis=0).T)
        in_maps.append({
            "x_t": x_perm_t, "wa_t": wa_t, "wqk_t": wqk_t,
            "wv_t": wv_t, "wp_t": wp_t, "ones2_t": _ONES2,
            "sel2_t": _SEL2, "sel64_t": _SEL64,
        })
    return in_maps


def assemble_output(results):
    out = np.empty((B, N, C), dtype=np.float32)
    for core in range(N_CORES):
        b, half = divmod(core, 2)
        out[b, half * NQ:(half + 1) * NQ] = results[core]["y_out"]
    return out


_ONES2 = np.zeros((128, 2), np.float32)
_ONES2[0:64, 0] = 1.0
_ONES2[64:128, 1] = 1.0
_SEL2 = np.zeros((2, 128), np.float32)
_SEL2[0, 0:64] = 1.0
_SEL2[1, 64:128] = 1.0
_SEL64 = np.zeros((65, 64), np.float32)
_SEL64[64, :] = 1.0

_NC_CACHE = {}


def kernel(**inputs) -> np.ndarray:
    if 1 not in _NC_CACHE:
        _NC_CACHE[1] = build_nc(reps=1)
    nc = _NC_CACHE[1]
    in_maps = prep_inputs(**inputs)
    res = run_bass_kernel_spmd(nc, in_maps, core_ids=list(range(N_CORES)))
    return assemble_output(res.results)
